# revision 1
# baseline (speedup 1.0000x reference)
"""Trainium2 Bass kernel for nn_ExpansionContrastModule.

Sharding: 8 cores = 4 batches x 2 H-halves (80 rows each). Bottom halves are
row-flipped on the host (conv weights H-flipped to match) so that image-pad
rows always sit at shard-top; the dwconv-product-sort stage is flip-invariant.

Within a core the 80 owned rows split into two 40-row sub-halves A/B mapped to
SBUF partitions 0:64 / 64:128 (64 channels each), computed in lockstep:
conv matmuls use block-diagonal duplicated weights (K=128, M=128) and all
vector ops run fully packed [128, N].

v2 design:
- Branch convs (k=3,5,7) run in fp8e4m3 with MatmulPerfMode.DoubleRow: two
  same-column taps (ki, ki+1) are packed per matmul via a manual pair AP
  (pair stride = x0 row stride = 176 bytes, 16B-aligned).
- in_conv runs fp8 DoubleRow too: the pair dim packs the two 128-channel
  groups of the 256-channel input.
- Matmul rhs spans rows flat (3 rows = 512 inner elems incl. row pads);
  garbage at pad columns lands in PSUM and is skipped by the strided ACT
  copy that applies the conv bias.
- Elementwise contrast stage split across DVE / Pool / ACT by tunable
  per-op-site engine maps; weighted-sum scales ride on ACT (identity with
  per-partition scale) or DVE tensor_scalar (4x mode).
- Final stage computes the shared BaseConv trunk once per row-block; cen for
  the final multiply streams in bf16; output is bf16, upcast on the host.

Restructured dwconv-contrast: with o1_m(p) = x(p) - x(p + v_m),
  o_m(p) = -o1_m(p) * o1_m(p - v_m),
so only 4 difference maps per branch are needed; the negation is absorbed by
using adjusted scales s'[g, j] = -scales3[g, 3-j] on the sorted products.

Geometry (per half, local coords): shard = 100 rows (10 pad/halo + 80 + 10
halo). Owned rows = shard 10..89; A owns 10..49, B owns 50..89.
x0 (in_conv out): 60 rows per half, row r <-> shard r (A) / 40+r (B),
row stride 176, real cols at [3, 163).
x_k (branch conv out): rows_x = 40+2d rows, row i <-> shard 10-d+i (A),
50-d+i (B); width Wx = 160+4d with real cols at [2d, 2d+160).
Shard rows 0..9 are always image-pad (flip trick) -> x0 rows 0..9 and
x_k rows 0..d-1 of the A half are memset to zero.
"""

import os

os.environ.setdefault("MYCRO_LOCAL_CACHE", "1")

import numpy as np
import ml_dtypes

import concourse.bass as bass
import concourse.bacc as bacc
import concourse.mybir as mybir
from concourse.ap import AP
from concourse.tile import TileContext
from concourse import bass_utils

W = 160
SH = 100          # shard rows
HALO = 10
OWNH = 40         # owned rows per half
C = 256
CH = 64           # trunk channels
WP0 = 176         # x0 row stride (16B aligned for fp8 pair APs)
X0R = 60
RF = 2            # final-stage block rows
KS = [1, 3, 5, 7]         # branch conv kernel sizes
DIL = [1, 3, 5, 7]        # branch dwconv dilations (= shift)
TAP_OFF = [0, 1, 10, 35]  # cumulative tap offsets into wtap
NTAP = 84

F32 = mybir.dt.float32
FP8 = mybir.dt.float8e4
BF16 = mybir.dt.bfloat16
ALU = mybir.AluOpType
ACTF = mybir.ActivationFunctionType
DR = mybir.MatmulPerfMode.DoubleRow

# --- tunables (env-overridable for sweeps) ---
RB = int(os.environ.get("K_RB", "20"))        # post-stage block rows
E_SUB = os.environ.get("K_ESUB", "vpvp")      # per direction j=0..3
E_MUL = os.environ.get("K_EMUL", "vppp")      # per direction j=0..3
E_SORT = os.environ.get("K_ESORT", "vvvvvvvvvv")  # 10 comparator ops
E_ADD = os.environ.get("K_EADD", "ppp")       # 3 wsum tree adds
E_ACC = os.environ.get("K_EACC", "vp")        # vmax, vsum accumulate
E_SCALE = os.environ.get("K_ESCALE", "aaaa")  # a=ACT, v=DVE ts, p=Pool ts
K_INCONV = os.environ.get("K_INCONV", "pool3")  # pool3 | act6
K_OUT = os.environ.get("K_OUT", "s")          # out8 dma engine (s/p/a)
K_CF = os.environ.get("K_CF", "s")            # cenf dma engine (s/p/a)


def _eng(nc, c):
    return {"v": nc.vector, "p": nc.gpsimd, "a": nc.scalar,
            "s": nc.sync}[c]


def _pair_ap(base, off, pair_stride, n):
    a = base
    return AP(a.tensor, a.offset + off,
              [[a.ap[0][0], 128], [pair_stride, 2], [1, n]])


def _flat_ap(base, off, n, npart=128):
    a = base
    return AP(a.tensor, a.offset + off, [[a.ap[0][0], npart], [1, n]])


def _rows_ap(base, off, rstride, nrows, ncols, npart=128):
    a = base
    return AP(a.tensor, a.offset + off,
              [[a.ap[0][0], npart], [rstride, nrows], [1, ncols]])


def build_nc(dbg=False):
    nc = bacc.Bacc("TRN2", target_bir_lowering=False, debug=False,
                   enable_asserts=False, num_devices=8)

    def dram(name, shape, dt, kind="ExternalInput"):
        return nc.dram_tensor(name, list(shape), dt, kind=kind).ap()

    cen8 = dram("cen8", (128, 2 * SH * W), FP8)       # [grp][shard pixels]
    cenf = dram("cenf", (128, 2 * 80 * W), BF16)      # owned rows, bf16
    win8 = dram("win8", (128, 2 * 64), FP8)
    wtap = dram("wtap", (128, NTAP * 128), FP8)
    wbc = dram("wbc", (128, 128), BF16)
    wfc = dram("wfc", (128, 2), BF16)
    ones1 = dram("ones1", (1, 128), BF16)
    bin_ = dram("bin", (128, 1), F32)
    cb = dram("cb", (128, 4), F32)
    sadj = dram("sadj", (128, 16), F32)
    bnsc = dram("bnsc", (128, 1), F32)
    bnbi = dram("bnbi", (128, 1), F32)
    fcb = dram("fcb", (1, 1), F32)
    out8 = dram("out8", (128, 2 * 80 * W), BF16, kind="ExternalOutput")

    with TileContext(nc) as tc:
        with tc.tile_pool(name="cpool", bufs=1) as cp, \
             tc.tile_pool(name="inpool", bufs=2) as ip, \
             tc.tile_pool(name="x0pool", bufs=1) as x0p, \
             tc.tile_pool(name="xpool", bufs=2) as xp, \
             tc.tile_pool(name="o1pool", bufs=1) as o1p, \
             tc.tile_pool(name="tpool", bufs=2) as tp, \
             tc.tile_pool(name="vpool", bufs=1) as vp, \
             tc.tile_pool(name="fpool", bufs=2) as fp, \
             tc.tile_pool(name="cfpool", bufs=3) as cfp, \
             tc.tile_pool(name="pspool", bufs=1, space="PSUM") as pp:

            # ---- constants to SBUF
            win_s = cp.tile_from(win8, name="win_s")
            wtap_s = cp.tile_from(wtap, name="wtap_s")
            wbc_s = cp.tile_from(wbc, name="wbc_s")
            wfc_s = cp.tile_from(wfc, name="wfc_s")
            ones_s = cp.tile_from(ones1, name="ones_s")
            bin_s = cp.tile_from(bin_, name="bin_s")
            cb_s = cp.tile_from(cb, name="cb_s")
            sadj_s = cp.tile_from(sadj, name="sadj_s")
            bnsc_s = cp.tile_from(bnsc, name="bnsc_s")
            bnbi_s = cp.tile_from(bnbi, name="bnbi_s")
            fcb_s = cp.tile_from(fcb, name="fcb_s")

            # ---- Phase A: in_conv -> x0 [128, 60*176] fp8
            x0 = x0p.tile([128, X0R * WP0], FP8, name="x0")
            x0v = x0[:, :].rearrange("p (r w) -> p r w", w=WP0)
            # zero col pads (left 3, right 13)
            nc.gpsimd.memset(x0v[:, :, 0:3], 0.0)
            nc.gpsimd.memset(x0v[:, :, 3 + W:WP0], 0.0)
            RT = 3                # rows per in_conv MM slice
            RTD = int(os.environ.get("K_RTD", "3"))  # rows per streamed input tile
            for ci, td in enumerate(range(0, X0R, RTD)):
                nd = RTD * W
                ct = ip.tile([128, 4 * nd], FP8, tag="cen", name=f"ct{td}")
                for h in range(2):
                    _eng(nc, os.environ.get("K_CTQ", "pp" if RTD == 3 else "sa")[h % 2]).dma_start(
                        out=ct[:, 2 * h * nd:2 * (h + 1) * nd]
                        .rearrange("p (g n) -> p g n", g=2),
                        in_=_rows_ap(cen8[:, 0:1], (h * OWNH + td) * W,
                                     SH * W, 2, nd))
                for t0 in range(0, RTD, RT):
                    n = RT * W
                    ps = pp.tile([128, 512], F32, tag="cvps", bufs=2,
                                 name="ps_in")
                    for h in range(2):
                        for g in range(2):
                            nc.tensor.matmul(
                                ps[h * 64:h * 64 + 64, 0:n],
                                lhsT=_flat_ap(win_s[:, 0:1], g * 64, 64),
                                rhs=_flat_ap(ct[:, 0:1],
                                             (2 * h + g) * nd + t0 * W, n),
                                start=(g == 0), stop=(g == 1))
                    nc.scalar.activation(
                        x0v[:, td + t0:td + t0 + RT, 3:3 + W],
                        ps[:, 0:n].rearrange("p (r w) -> p r w", w=W),
                        ACTF.Identity, bias=bin_s[:, 0:1])
            # zero image-pad rows of A half (shard rows 0..9)
            nc.gpsimd.memset(x0[0:64, 0:HALO * WP0], 0.0)

            # ---- vmax / vsum accumulators [128, 40*160] bf16
            vmax = vp.tile([128, OWNH * W], BF16, name="vmax")
            vsum = vp.tile([128, OWNH * W], BF16, name="vsum")

            # ---- final stage emitter: one RF-row block ----
            def emit_final(f):
                n = RF * W
                sl = slice(f * W, (f + RF) * W)
                # m = relu(vmax + 0.25*vsum)
                mt = fp.tile([128, n], BF16, tag="mt", name="mt")[:, :]
                nc.vector.tensor_scalar_mul(mt, vsum[:, sl], 0.25)
                _eng(nc, os.environ.get("K_MTA", "v")).tensor_tensor(mt, mt, vmax[:, sl], ALU.add)
                nc.vector.tensor_scalar_max(mt, mt, 0.0)
                zps = pp.tile([128, n], F32, tag="zps", bufs=2, name="zps")
                nc.tensor.matmul(zps[:, :], lhsT=wbc_s[:, :], rhs=mt,
                                 start=True, stop=True)
                # BN + SiLU: silu(v) = v * sigmoid(v), v = scale*z + bias
                zlin = fp.tile([128, n], BF16, tag="zlin", name="zlin")[:, :]
                if os.environ.get("K_ZLIN", "a") == "v":
                    nc.vector.tensor_scalar(zlin, zps[:, :], bnsc_s[:, 0:1],
                                            bnbi_s[:, 0:1], op0=ALU.mult,
                                            op1=ALU.add)
                else:
                    nc.scalar.activation(zlin, zps[:, :], ACTF.Identity,
                                         bias=bnbi_s[:, 0:1],
                                         scale=bnsc_s[:, 0:1])
                zsig = fp.tile([128, n], BF16, tag="zsig", name="zsig")[:, :]
                nc.scalar.activation(zsig, zps[:, :], ACTF.Sigmoid,
                                     bias=bnbi_s[:, 0:1], scale=bnsc_s[:, 0:1])
                zt = fp.tile([128, n], BF16, tag="zt", name="zt")[:, :]
                nc.vector.tensor_mul(zt, zlin, zsig)
                for h in range(2):
                    lps = pp.tile([1, n], F32, tag="lps", bufs=2,
                                  name="lps")
                    nc.tensor.matmul(lps[:, :], lhsT=wfc_s[:, h:h + 1],
                                     rhs=zt, start=True, stop=True)
                    msk = fp.tile([1, n], BF16, tag=f"msk{h}", name="msk")
                    nc.scalar.activation(msk[:, :], lps[:, :], ACTF.Sigmoid,
                                         bias=fcb_s[0:1, 0:1])
                    mb = pp.tile([128, n], F32, tag="mb", bufs=2,
                                 name="mb")
                    nc.tensor.matmul(mb[:, :], lhsT=ones_s[:, :],
                                     rhs=msk[:, :], start=True, stop=True)
                    # mbs = mask + 1 in SBUF bf16 (GPSIMD cannot read PSUM)
                    mbs = fp.tile([128, n], BF16, tag="mbs",
                                  name="mbs")[:, :]
                    nc.vector.tensor_scalar_add(mbs, mb[:, :], 1.0)
                    ct = cfp.tile([128, 2 * n], BF16, tag=f"cf{h}",
                                  name="cent")
                    src = (h * OWNH + f) * W
                    _eng(nc, K_CF).dma_start(
                        out=ct[:, :].rearrange("p (g n) -> p g n", g=2),
                        in_=_rows_ap(cenf[:, 0:1], src, 80 * W, 2, n))
                    ot = fp.tile([128, 2 * n], BF16, tag=f"ot{h}", name="ot")
                    for g, ge in enumerate(os.environ.get("K_OTM", "pp")):
                        _eng(nc, ge).tensor_mul(
                            ot[:, g * n:(g + 1) * n], mbs,
                            ct[:, g * n:(g + 1) * n])
                    dst = (h * OWNH + f) * W
                    _eng(nc, K_OUT).dma_start(
                        out=_rows_ap(out8[:, 0:1], dst, 80 * W, 2, n),
                        in_=ot[:, :].rearrange("p (g n) -> p g n", g=2))

            _KORD = [int(c) for c in os.environ.get("K_KORD", "0123")]
            for kidx, k in enumerate(_KORD):
                first_k, last_k = (kidx == 0), (kidx == 3)
                d = DIL[k]
                ksz = KS[k]
                pad = ksz // 2
                rows_x = OWNH + 2 * d
                Wx = W + 4 * d
                xk = xp.tile([128, rows_x * Wx], BF16, tag="x", name=f"x{k}")
                xv = xk[:, :].rearrange("p (r w) -> p r w", w=Wx)
                # zero col pads
                nc.gpsimd.memset(xk[:, 0:2 * d], 0.0)
                nc.gpsimd.memset(
                    xk[:, 2 * d + W:2 * d + W + (rows_x - 1) * Wx]
                    .rearrange("p (r w) -> p r w", w=Wx)[:, :, 0:4 * d], 0.0)
                nc.gpsimd.memset(
                    xk[:, (rows_x - 1) * Wx + 2 * d + W:rows_x * Wx], 0.0)

                # branch conv: fp8 DR pairs (same column, ki&ki+1) + singles
                for rt in range(0, rows_x, RT):
                    nr = min(RT, rows_x - rt)
                    ni = (nr - 1) * WP0 + W
                    ps = pp.tile([128, 512], F32, tag="cvps", bufs=2,
                                 name=f"ps{k}")
                    nmm = ksz * ((ksz // 2) + (ksz % 2))
                    ti = 0
                    for kj in range(ksz):
                        dx = kj - pad
                        col_off = 3 + dx
                        for ki0 in range(0, ksz - 1, 2):
                            r0 = HALO - d + rt + (ki0 - pad)
                            tap = TAP_OFF[k] + kj * ksz + ki0
                            nc.tensor.matmul(
                                ps[:, 0:ni],
                                lhsT=_pair_ap(wtap_s[:, 0:1], tap * 128,
                                              128, 128),
                                rhs=_pair_ap(x0[:, 0:1], r0 * WP0 + col_off,
                                             WP0, ni),
                                start=(ti == 0), stop=(ti == nmm - 1),
                                perf_mode=DR)
                            ti += 1
                        if ksz % 2:
                            ki = ksz - 1
                            r0 = HALO - d + rt + (ki - pad)
                            tap = TAP_OFF[k] + kj * ksz + ki
                            nc.tensor.matmul(
                                ps[:, 0:ni],
                                lhsT=_flat_ap(wtap_s[:, 0:1], tap * 128, 128),
                                rhs=_flat_ap(x0[:, 0:1], r0 * WP0 + col_off,
                                             ni),
                                start=(ti == 0), stop=(ti == nmm - 1))
                            ti += 1
                    nc.scalar.activation(
                        xv[:, rt:rt + nr, 2 * d:2 * d + W],
                        _rows_ap(ps[:, 0:1], 0, WP0, nr, W),
                        ACTF.Identity, bias=cb_s[:, k:k + 1])
                # zero image-pad rows of A half: x rows 0..d-1
                nc.gpsimd.memset(xk[0:64, 0:d * Wx], 0.0)

                # ---- post stage: blocks of RB owned rows
                WPK = W + 2 * d
                vs = [(-d, -d), (-d, 0), (-d, d), (0, -d)]
                # per-direction o1 extents: (rows, cols, x col of o1 col 0)
                o1geo = [(RB + d, W + d, 2 * d), (RB + d, W, 2 * d),
                         (RB + d, W + d, d), (RB, W + d, 2 * d)]
                for b in range(0, OWNH, RB):
                    ts = []
                    for j, (dy, dx) in enumerate(vs):
                        nrj, ncj, cxj = o1geo[j]
                        o1 = o1p.tile([128, nrj * ncj], BF16, tag=f"o1{j}",
                                      name=f"o1_{k}_{b}_{j}")
                        o1v = o1[:, :].rearrange("p (r w) -> p r w", w=ncj)
                        xr0 = d + b   # x row of o1 row 0
                        _eng(nc, E_SUB[j]).tensor_sub(
                            o1v[:, :, :],
                            xv[:, xr0:xr0 + nrj, cxj:cxj + ncj],
                            xv[:, xr0 + dy:xr0 + dy + nrj,
                               cxj + dx:cxj + dx + ncj])
                        tj = tp.tile([128, RB * W], BF16, tag=f"t{j}",
                                     name=f"t{k}_{b}_{j}")
                        c1 = 2 * d - cxj           # op1 col base (x col 2d)
                        c2 = 2 * d - dx - cxj      # op2 col base
                        _eng(nc, E_MUL[j]).tensor_mul(
                            tj[:, :].rearrange("p (r w) -> p r w", w=W),
                            o1v[:, 0:RB, c1:c1 + W],
                            o1v[:, -dy:-dy + RB, c2:c2 + W])
                        ts.append(tj)
                    t0_, t1_, t2_, t3_ = [t[:, :] for t in ts]
                    e1 = tp.tile([128, RB * W], BF16, tag="e1",
                                 bufs=int(os.environ.get("K_E1B", "2")),
                                 name=f"e{k}_{b}")[:, :]
                    # 5-comparator sort network, in-place (ascending finals:
                    # t3_=o(1), t1_=o(2), t0_=o(3), e1=o(4))
                    S = [_eng(nc, c) for c in E_SORT]
                    S[0].tensor_tensor(e1, t0_, t1_, ALU.max)
                    S[1].tensor_tensor(t0_, t0_, t1_, ALU.min)
                    S[2].tensor_tensor(t1_, t2_, t3_, ALU.max)
                    S[3].tensor_tensor(t2_, t2_, t3_, ALU.min)
                    S[4].tensor_tensor(t3_, t0_, t2_, ALU.min)
                    S[5].tensor_tensor(t0_, t0_, t2_, ALU.max)
                    S[6].tensor_tensor(t2_, e1, t1_, ALU.min)
                    S[7].tensor_tensor(e1, e1, t1_, ALU.max)
                    S[8].tensor_tensor(t1_, t0_, t2_, ALU.min)
                    S[9].tensor_tensor(t0_, t0_, t2_, ALU.max)
                    # weighted sum: scales in-place, tree-add
                    for sj, tgt in zip(range(4), (t3_, t1_, t0_, e1)):
                        col = sadj_s[:, 4 * k + sj:4 * k + sj + 1]
                        ec = E_SCALE[sj]
                        if ec == "a":
                            nc.scalar.activation(tgt, tgt, ACTF.Identity,
                                                 scale=col)
                        else:
                            _eng(nc, ec).tensor_scalar_mul(tgt, tgt, col)
                    vmx = vmax[:, b * W:(b + RB) * W]
                    vsm = vsum[:, b * W:(b + RB) * W]
                    A = [_eng(nc, c) for c in E_ADD]
                    A[0].tensor_tensor(t3_, t3_, t1_, ALU.add)
                    A[1].tensor_tensor(t0_, t0_, e1, ALU.add)
                    if first_k:
                        A[2].tensor_tensor(vsm, t3_, t0_, ALU.add)
                        nc.vector.tensor_copy(vmx, vsm)
                    else:
                        A[2].tensor_tensor(t1_, t3_, t0_, ALU.add)
                        C0 = [_eng(nc, c) for c in E_ACC]
                        C0[0].tensor_tensor(vmx, vmx, t1_, ALU.max)
                        C0[1].tensor_tensor(vsm, vsm, t1_, ALU.add)
                    if last_k:
                        # interleave final-stage emission with the last
                        # branch's post blocks so engine FIFOs can overlap
                        for f in range(b, b + RB, RF):
                            emit_final(f)

    nc.compile()
    nc.finalize()
    return nc


_NC_CACHE = None


def _get_nc():
    global _NC_CACHE
    if _NC_CACHE is None:
        _NC_CACHE = build_nc()
    return _NC_CACHE


def _prep_core_inputs(cen_b, flip, wts):
    """cen_b: (256, 160, 160) fp32 for this batch; flip: bottom half?"""
    (w_in, b_in, convs, scales_adj, bc_w, bn_scale, bn_bias,
     fc_w, fc_b) = wts
    lo = (1 if flip else 0) * 80 - HALO
    sh = np.zeros((C, SH, W), np.float32)
    r0, r1 = max(0, lo), min(160, lo + SH)
    sh[:, r0 - lo:r1 - lo] = cen_b[:, r0:r1]
    if flip:
        sh = sh[:, ::-1]
    sh = np.ascontiguousarray(sh)

    f8 = ml_dtypes.float8_e4m3
    bf = ml_dtypes.bfloat16

    wtap = np.zeros((128, NTAP * 128), f8)
    for k in range(4):
        ksz = KS[k]
        cw = convs[k][0]
        if flip:
            cw = cw[:, :, ::-1, :]
        for kj in range(ksz):
            for ki in range(ksz):
                t = TAP_OFF[k] + kj * ksz + ki
                blk = cw[:, :, ki, kj].T.astype(f8)  # [ci, co]
                wtap[0:64, t * 128:t * 128 + 64] = blk
                wtap[64:128, t * 128 + 64:t * 128 + 128] = blk

    win8 = np.zeros((128, 2 * 64), f8)
    win8[:, 0:64] = w_in[:, 0:128].T.astype(f8)
    win8[:, 64:128] = w_in[:, 128:256].T.astype(f8)

    wbc = np.zeros((128, 128), bf)
    wbc[0:64, 0:64] = bc_w.T.astype(bf)
    wbc[64:128, 64:128] = bc_w.T.astype(bf)

    wfc = np.zeros((128, 2), bf)
    wfc[0:64, 0] = fc_w.astype(bf)
    wfc[64:128, 1] = fc_w.astype(bf)

    sh8 = sh.astype(f8).reshape(2, 128, SH * W)
    cen8 = np.concatenate([sh8[0], sh8[1]], axis=1)  # [128, 2*SH*W]
    shf = sh[:, HALO:HALO + 80].astype(bf).reshape(2, 128, 80 * W)
    cenf = np.concatenate([shf[0], shf[1]], axis=1)

    dup = lambda v: np.concatenate([v, v]).astype(np.float32).reshape(128, -1)
    m = {
        "cen8": cen8,
        "cenf": cenf,
        "win8": win8,
        "wtap": wtap,
        "wbc": wbc,
        "wfc": wfc,
        "ones1": np.ones((1, 128), bf),
        "bin": dup(b_in),
        "cb": np.concatenate([np.stack([cb for _, cb in convs], 1)] * 2, 0)
              .astype(np.float32),
        "sadj": np.concatenate([scales_adj.reshape(64, 16)] * 2, 0)
                .astype(np.float32),
        "bnsc": dup(bn_scale),
        "bnbi": dup(bn_bias),
        "fcb": np.full((1, 1), fc_b, np.float32),
    }
    return m


def make_in_maps(inputs):
    cen = np.asarray(inputs["cen"], np.float32)
    w_in = np.asarray(inputs["in_conv_w"], np.float32).reshape(CH, C)
    convs = [(np.asarray(inputs[f"conv{k}_w"], np.float32),
              np.asarray(inputs[f"conv{k}_b"], np.float32))
             for k in (1, 3, 5, 7)]
    # s'[g, j] = -scales3[g, 3-j]
    sadj = -np.asarray(inputs["scales3"], np.float32)[:, ::-1]    # (64, 4)
    sadj4 = np.repeat(sadj[:, None, :], 4, axis=1)                # (64, 4, 4)
    bn_scale = (np.asarray(inputs["bn_gamma"]) /
                np.sqrt(np.asarray(inputs["bn_var"]) + 1e-5)).astype(np.float32)
    bn_bias = (np.asarray(inputs["bn_beta"]) -
               np.asarray(inputs["bn_mean"]) * bn_scale).astype(np.float32)
    wts = (w_in, np.asarray(inputs["in_conv_b"], np.float32), convs, sadj4,
           np.asarray(inputs["bc_w"], np.float32).reshape(CH, CH),
           bn_scale, bn_bias,
           np.asarray(inputs["fc_w"], np.float32).reshape(CH),
           float(np.asarray(inputs["fc_b"])[0]))
    in_maps = []
    for core in range(8):
        b, half = core // 2, core % 2
        in_maps.append(_prep_core_inputs(cen[b], half == 1, wts))
    return in_maps


def kernel(**inputs):
    in_maps = make_in_maps(inputs)
    nc = _get_nc()
    res = bass_utils.run_bass_kernel_spmd(nc, in_maps,
                                          core_ids=list(range(8)))
    out = np.empty((4, C, 160, W), np.float32)
    for core in range(8):
        b, half = core // 2, core % 2
        o8 = np.asarray(res.results[core]["out8"]).reshape(128, 2, 80, W)
        o = np.concatenate([o8[:, 0], o8[:, 1]], 0).astype(np.float32)
        if half == 1:
            o = o[:, ::-1]
        out[b, :, half * 80:(half + 1) * 80] = o
    return out

